# revision 13
# baseline (speedup 1.0000x reference)
"""Trainium2 Bass kernel for KernelAttentionEncoder.

Reference math (per batch element b, N=2048 nodes, D=O=128, H=3 heads):
  d2[i,j]   = ||c_i - c_j||^2
  logits    = clip(-d2 / sigma_h^2, -20, 20), masked pairs -> -1e9
  attn      = softmax_j(logits)
  values_h  = node_features @ Wv_h
  head_h    = attn_h @ values_h
  out       = concat_h(head_h) @ Wo + bo, masked rows zeroed

Strategy: data-parallel over B=8 across the 8 NeuronCores (one batch element
per core). Per core, a fused flash-style kernel that never materializes the
NxN matrices in HBM:

  - d2 tile [128 j, 512 i] via one K=5 fp32 matmul using the Gram expansion:
    lhsT rows [cx,cy,cz,|c|^2,1] x rhs rows [-2cx,-2cy,-2cz,1,|c|^2].
  - E_h = exp(-d2/sigma_h^2) straight from PSUM on the scalar engine
    (ACT exp with scale). The reference's clip at -20 only affects weights
    below exp(-20)~2e-9; omitting it changes the output by ~1e-6 relative.
  - P@V in weights-transposed orientation: psum2_h[o, i] += V_h[j,:]^T E[j,i]
    accumulated over j tiles (fp32r matmuls run at full PE rate at N=512).
    Masking is exact: V rows are zeroed for masked (padded) j, so masked
    columns contribute exactly 0 to both numerator and denominator.
  - Softmax denominators S_h[i] broadcast to all partitions via a matmul
    whose lhsT is colmask replicated across 128 columns: psumS_h[*, i] =
    sum_j colmask_j E[j,i]. Normalize multiT_h = psum2_h * 1/psumS_h.
  - Output projection: one psum accumulates sum_h multiT_h^T @ Wo_h
    (fp32), then + bo and row-mask on the way to SBUF.
"""

import numpy as np
from contextlib import ExitStack

import concourse.bass as bass
import concourse.bacc as bacc
import concourse.tile as tile
import concourse.mybir as mybir
from concourse import bass_utils

F32 = mybir.dt.float32
F32R = mybir.dt.float32r
BF16 = mybir.dt.bfloat16

B, N, D, O, H = 8, 2048, 128, 128, 3
SIGMAS = (1.0, 2.0, 4.0)
NJT = N // 128          # 16 j-tiles of 128 (contraction/partition dim)
NIB = 4                 # i-blocks of 512
IB = 512
NSL = IB // 128         # 4 i-slices of 128 per block
NIT = N // 128          # 16 i-tiles total

_CACHE = {}


def _build_nc():
    nc = bacc.Bacc("TRN2", target_bir_lowering=False, debug=False, num_devices=B)

    d_nfT = nc.dram_tensor("nfT", [D, N], F32, kind="ExternalInput")
    d_cj13 = nc.dram_tensor("cj13", [24, N], BF16, kind="ExternalInput")
    d_ci13 = nc.dram_tensor("ci13", [24, N], BF16, kind="ExternalInput")
    d_wv = nc.dram_tensor("wv", [H, D, O], F32, kind="ExternalInput")
    d_wo = nc.dram_tensor("wo", [H, O, O], F32, kind="ExternalInput")
    d_bob = nc.dram_tensor("bob", [128, O], F32, kind="ExternalInput")
    d_colm = nc.dram_tensor("colm", [128, NJT], F32, kind="ExternalInput")
    d_rowm = nc.dram_tensor("rowm", [128, NIT], F32, kind="ExternalInput")
    d_out = nc.dram_tensor("out", [N, O], F32, kind="ExternalOutput")

    inv_s2 = [1.0 / (s * s) for s in SIGMAS]

    with tile.TileContext(nc) as tc, ExitStack() as ctx:
        cpool = ctx.enter_context(tc.tile_pool(name="const", bufs=1))
        vpool = ctx.enter_context(tc.tile_pool(name="v1", bufs=1))
        epool = ctx.enter_context(tc.tile_pool(name="e", bufs=8))
        mpool = ctx.enter_context(tc.tile_pool(name="mt", bufs=1))
        rpool = ctx.enter_context(tc.tile_pool(name="recs", bufs=4))
        outp = ctx.enter_context(tc.tile_pool(name="outp", bufs=4))
        ps_d2 = ctx.enter_context(tc.tile_pool(name="ps_d2", bufs=2, space="PSUM"))
        ps_acc = ctx.enter_context(tc.tile_pool(name="ps_acc", bufs=3, space="PSUM"))
        ps_s = ctx.enter_context(tc.tile_pool(name="ps_s", bufs=3, space="PSUM"))

        # ---- persistent SBUF tiles (distinct tags => distinct allocations)
        def ctile(nm, shape, dt=F32):
            return cpool.tile(shape, dt, name=nm, tag=nm)

        t_nfT = ctile("t_nfT", [128, N])
        t_cj13 = ctile("t_cj13", [24, N], BF16)
        t_ci13 = ctile("t_ci13", [24, N], BF16)
        t_wv = ctile("t_wv", [128, H * O])
        t_wo = ctile("t_wo", [128, H * O])
        t_bob = ctile("t_bob", [128, O])
        t_colm = ctile("t_colm", [128, NJT])
        t_rowm = ctile("t_rowm", [128, NIT])
        t_ones = ctile("t_ones", [128, 128])
        t_crep = ctile("t_crep", [128, NJT * 128], F32R)

        nc.sync.dma_start(t_nfT[:], d_nfT.ap())
        nc.sync.dma_start(t_cj13[:], d_cj13.ap())
        nc.sync.dma_start(t_ci13[:], d_ci13.ap())
        for h in range(H):
            nc.sync.dma_start(t_wv[:, h * O:(h + 1) * O], d_wv.ap()[h])
            nc.sync.dma_start(t_wo[:, h * O:(h + 1) * O], d_wo.ap()[h])
        nc.sync.dma_start(t_bob[:], d_bob.ap())
        nc.sync.dma_start(t_colm[:], d_colm.ap())
        nc.sync.dma_start(t_rowm[:], d_rowm.ap())
        nc.vector.memset(t_ones[:], 1.0)
        # colmask_j replicated across 128 columns, per j-tile (f32r lhsT
        # for the row-sum matmuls)
        for jt in range(NJT):
            nc.vector.tensor_scalar(
                t_crep[:, jt * 128:(jt + 1) * 128], t_ones[:],
                t_colm[:, jt:jt + 1], None, mybir.AluOpType.mult,
            )

        # ---- V phase: V'_h[jt] [128 j, 128 o] f32r = (nfT_jt^T @ Wv_h) * colmask_j
        v1 = [[None] * NJT for _ in range(H)]
        for jt in range(NJT):
            for h in range(H):
                pv = ps_acc.tile([128, O], F32, name="pv", tag="acc")
                nc.tensor.matmul(
                    pv[:],
                    t_nfT[:, jt * 128:(jt + 1) * 128],
                    t_wv[:, h * O:(h + 1) * O],
                    start=True, stop=True,
                )
                vt = vpool.tile([128, O], F32R, name=f"v{h}_{jt}", tag=f"v{h}_{jt}")
                nc.vector.tensor_scalar(
                    vt[:], pv[:], t_colm[:, jt:jt + 1], None,
                    mybir.AluOpType.mult,
                )
                v1[h][jt] = vt

        # ---- main loop over i-blocks
        for it in range(NIB):
            i0 = it * IB
            psum2 = [
                ps_acc.tile([128, IB], F32, name=f"p2_{h}", tag="acc")
                for h in range(H)
            ]
            psumS = [
                ps_s.tile([128, IB], F32, name=f"ps_{h}", tag="s")
                for h in range(H)
            ]
            for jt in range(NJT):
                pd2 = ps_d2.tile([128, IB], F32, name="pd2", tag="d2")
                nc.tensor.matmul(
                    pd2[:],
                    t_cj13[:, jt * 128:(jt + 1) * 128],
                    t_ci13[:, i0:i0 + IB],
                    start=True, stop=True,
                )
                for h in range(H):
                    et = epool.tile([128, IB], F32R, name="et", tag="et")
                    nc.scalar.activation(
                        et[:], pd2[:],
                        mybir.ActivationFunctionType.Exp,
                        scale=-inv_s2[h],
                    )
                    nc.tensor.matmul(
                        psum2[h][:], v1[h][jt][:], et[:],
                        start=(jt == 0), stop=(jt == NJT - 1),
                    )
                    nc.tensor.matmul(
                        psumS[h][:], t_crep[:, jt * 128:(jt + 1) * 128], et[:],
                        start=(jt == 0), stop=(jt == NJT - 1),
                    )

            # ---- normalize: multiT_h = psum2_h / S_h  (S broadcast on all
            # partitions of psumS_h)
            multiT = []
            for h in range(H):
                rs = rpool.tile([128, IB], F32, name="rs", tag="rs")
                nc.vector.reciprocal(rs[:], psumS[h][:])
                mt = mpool.tile([128, IB], F32, name=f"mt{h}", tag=f"mt{h}", bufs=2)
                nc.vector.tensor_tensor(
                    mt[:], psum2[h][:], rs[:], mybir.AluOpType.mult
                )
                multiT.append(mt)

            # ---- output projection per i-slice: psum3 = sum_h multiT_h^T Wo_h
            for s in range(NSL):
                ti = it * NSL + s
                p3 = ps_acc.tile([128, O], F32, name="p3", tag="acc")
                for h in range(H):
                    nc.tensor.matmul(
                        p3[:],
                        multiT[h][:, s * 128:(s + 1) * 128],
                        t_wo[:, h * O:(h + 1) * O],
                        start=(h == 0), stop=(h == H - 1),
                    )
                ab = outp.tile([128, O], F32, name="ab", tag="ab")
                nc.vector.tensor_tensor(
                    ab[:], p3[:], t_bob[:], mybir.AluOpType.add
                )
                ot = outp.tile([128, O], F32, name="ot", tag="ot")
                nc.vector.tensor_scalar(
                    ot[:], ab[:], t_rowm[:, ti:ti + 1], None,
                    mybir.AluOpType.mult,
                )
                nc.sync.dma_start(d_out.ap()[ti * 128:(ti + 1) * 128, :], ot[:])

    nc.compile()
    return nc


def _prepare_core_inputs(nf_b, c_b, mask_b, Wv, Wo, bo):
    import ml_dtypes

    bf16 = ml_dtypes.bfloat16

    def split3(x):
        """x (fp32) -> 3 bf16 parts summing to x within ~2^-27 relative."""
        h = x.astype(bf16)
        r1 = x - h.astype(np.float32)
        m = r1.astype(bf16)
        l = (r1 - m.astype(np.float32)).astype(bf16)
        return h, m, l

    c = c_b.astype(np.float32)                      # [N, 3]
    c2 = (c * c).sum(axis=1, dtype=np.float32)      # [N]
    ch, cm, cl = split3(c)                          # [N, 3] each
    c2h, c2m, c2l = split3(c2)                      # [N] each
    one = np.ones((1, N), bf16)
    hT, mT, lT = ch.T, cm.T, cl.T                   # [3, N]

    def neg2(x):
        return (-2.0 * x.astype(np.float32)).astype(bf16)  # exact scaling

    # d2[j,i] = |cj|^2 + |ci|^2 - 2 cj.ci with cj.ci expanded over the
    # split pairs (h,h),(h,m),(m,h),(h,l),(l,h),(m,m); dropped terms are
    # O(2^-27). 18 cross rows + 3 |cj|^2 rows + 3 |ci|^2 rows = 24.
    cj13 = np.concatenate(
        [hT, hT, mT, hT, lT, mT,
         c2h[None], c2m[None], c2l[None], one, one, one]
    ).astype(bf16)
    ci13 = np.concatenate(
        [neg2(hT), neg2(mT), neg2(hT), neg2(lT), neg2(hT), neg2(mT),
         one, one, one, c2h[None], c2m[None], c2l[None]]
    ).astype(bf16)
    valid = (~mask_b).astype(np.float32)
    vT = np.ascontiguousarray(valid.reshape(NJT, 128).T)  # [128, 16]
    return {
        "nfT": np.ascontiguousarray(nf_b.astype(np.float32).T),
        "cj13": np.ascontiguousarray(cj13),
        "ci13": np.ascontiguousarray(ci13),
        "wv": np.ascontiguousarray(Wv.astype(np.float32)),
        "wo": np.ascontiguousarray(Wo.astype(np.float32).reshape(H, O, O)),
        "bob": np.ascontiguousarray(
            np.broadcast_to(bo.astype(np.float32), (128, O))
        ),
        "colm": vT,
        "rowm": vT.copy(),
    }


def kernel(node_features, coordinates, masked_elements, Wv, Wo, bo):
    node_features = np.asarray(node_features)
    coordinates = np.asarray(coordinates)
    masked_elements = np.asarray(masked_elements)
    Wv, Wo, bo = np.asarray(Wv), np.asarray(Wo), np.asarray(bo)

    if "nc" not in _CACHE:
        _CACHE["nc"] = _build_nc()
    nc = _CACHE["nc"]

    in_maps = [
        _prepare_core_inputs(
            node_features[b], coordinates[b], masked_elements[b], Wv, Wo, bo
        )
        for b in range(B)
    ]
    res = bass_utils.run_bass_kernel_spmd(nc, in_maps, core_ids=list(range(B)))
    out = np.stack([res.results[b]["out"] for b in range(B)])
    return out.astype(np.float32)
